# revision 11
# baseline (speedup 1.0000x reference)
"""Llama attention (B=2, S=2048, E=4096, H=32) on 8 trn2 NeuronCores.

Strategy (tensor-parallel over heads, 4 heads/core):
  - RoPE here is position-independent (cos/sin are [H, D/2], broadcast over
    seq), so it is a fixed per-head linear rotation folded into wq/wk on the
    host.  The 1/sqrt(D) score scale is folded into wq as well.
  - Scores are computed transposed (S^T = K^T-tile @ Q^T) and the attention
    output as O^T = V-tile @ P^T, so the device kernel is pure matmuls +
    exp with zero on-device transposes.  Softmax needs no max-subtraction
    (scores are bounded ~ +-8 here; fp32 exp cannot overflow).
  - Softmax denominators via an M=1 ones-matmul on the PE; the reciprocal is
    broadcast across partitions with a K=1 ones-matmul.
  - All matmuls run as float32r (full fp32 data, 1 cycle/row for N>=256).
  - Per-core output is a partial Y (row-sharded wo); host sums the 8 partials.
"""

import sys

sys.path.insert(0, "/opt/trn_rl_repo")

import numpy as np

B, S, E, H = 2, 2048, 4096, 32
D = 128            # head dim
NCORES = 8
HL = H // NCORES   # heads per core = 4
W = HL * D         # per-core projection width = 512
T = B * S          # 4096 tokens
KB = 8             # contraction blocks over E (512 each)
KK = 4             # 128-row k-tiles per block
NCH = 4            # 512-token chunks per batch
CH = 512

_CACHE = {}


def _build_nc():
    import concourse.bass as bass  # noqa: F401
    import concourse.mybir as mybir
    import concourse.tile as tile
    from concourse import bacc

    fp32 = mybir.dt.float32
    fp32r = mybir.dt.float32r
    EXP = mybir.ActivationFunctionType.Exp

    nc = bacc.Bacc("TRN2", target_bir_lowering=False, debug=False)

    xT_d = nc.dram_tensor("xT", [E, T], fp32r, kind="ExternalInput")
    wqk_d = nc.dram_tensor("wqk", [KB, 128, KK, 2 * W], fp32r, kind="ExternalInput")
    wv_d = nc.dram_tensor("wv", [KB, 128, KK, W], fp32r, kind="ExternalInput")
    wo_d = nc.dram_tensor("wo", [128, KK, E], fp32r, kind="ExternalInput")
    y_d = nc.dram_tensor("y", [T, E], fp32, kind="ExternalOutput")

    xview = xT_d.rearrange("(kb kk p) t -> kb p kk t", kk=KK, p=128)

    with nc.allow_low_precision(reason="fp32r feeds PE; rounding is intended"), \
         tile.TileContext(nc) as tc:
        with tc.tile_pool(name="const", bufs=1) as constp, \
             tc.tile_pool(name="g_wo", bufs=2) as wop, \
             tc.tile_pool(name="g_yt", bufs=2) as yp, \
             tc.tile_pool(name="g_rec", bufs=1) as rp, \
             tc.tile_pool(name="g_gp", bufs=4, space="PSUM") as gpp, \
             tc.tile_pool(name="g_pS", bufs=2, space="PSUM") as pSp, \
             tc.tile_pool(name="g_po", bufs=1, space="PSUM") as pop, \
             tc.tile_pool(name="g_sum", bufs=1, space="PSUM") as psump:
            ones_f = constp.tile([128, 1], fp32, tag="ones_f")
            nc.vector.memset(ones_f[:], 1.0)
            ones_rf = constp.tile([1, 128], fp32, tag="ones_rf")
            nc.vector.memset(ones_rf[:], 1.0)
            ones_col = constp.tile([128, 1], fp32r, tag="ones_col")
            nc.vector.tensor_copy(ones_col[:], ones_f[:])
            ones_row = constp.tile([1, 128], fp32r, tag="ones_row")
            nc.vector.tensor_copy(ones_row[:], ones_rf[:])
            zbias = constp.tile([128, 1], fp32, tag="zbias")
            nc.vector.memset(zbias[:], 0.0)

            for b in range(B):
                with tc.tile_pool(name=f"ot{b}", bufs=1) as otp:
                    OT = [otp.tile([128, S], fp32r, tag=f"ot{i}", name=f"ot{i}") for i in range(HL)]
                    with tc.tile_pool(name=f"qkv{b}", bufs=1) as qkvp:
                        QT = [qkvp.tile([128, S], fp32r, tag=f"qt{i}", name=f"qt{i}") for i in range(HL)]
                        KT = [qkvp.tile([128, S], fp32r, tag=f"kt{i}", name=f"kt{i}") for i in range(HL)]
                        V = [qkvp.tile([128, W], fp32r, tag=f"v{i}", name=f"v{i}") for i in range(4 * NCH)]

                        # ---------------- phase 1: projections ----------------
                        with tc.tile_pool(name=f"p1w{b}", bufs=2) as wpool, \
                             tc.tile_pool(name=f"p1wv{b}", bufs=1) as wvpool, \
                             tc.tile_pool(name=f"p1x{b}", bufs=2) as xpool:
                            for kb in range(KB):
                                wqk_t = wpool.tile([128, KK, 2 * W], fp32r, tag="wqk")
                                nc.sync.dma_start(wqk_t[:], wqk_d[kb])
                                wv_t = wvpool.tile([128, KK, W], fp32r, tag="wv")
                                nc.sync.dma_start(wv_t[:], wv_d[kb])
                                for n in range(NCH):
                                    tok0 = b * S + n * CH
                                    xc = xpool.tile([128, KK, CH], fp32r, tag="xc")
                                    nc.sync.dma_start(
                                        xc[:], xview[kb, :, :, tok0:tok0 + CH]
                                    )
                                    for proj in range(2):  # 0 -> QT, 1 -> KT
                                        for mi in range(HL):
                                            ps = gpp.tile([128, CH], fp32, tag="gp", name="ps")
                                            c0 = proj * W + mi * 128
                                            for kk in range(KK):
                                                nc.tensor.matmul(
                                                    ps[:],
                                                    wqk_t[:, kk, c0:c0 + 128],
                                                    xc[:, kk, :],
                                                    start=(kk == 0),
                                                    stop=(kk == KK - 1),
                                                )
                                            dst = (QT if proj == 0 else KT)[mi][:, n * CH:(n + 1) * CH]
                                            if kb == 0:
                                                nc.vector.tensor_copy(dst, ps[:])
                                            else:
                                                nc.vector.tensor_add(dst, dst, ps[:])
                                    for mt in range(4):  # V: token tiles in chunk
                                        ps = gpp.tile([128, W], fp32, tag="gp", name="psv")
                                        for kk in range(KK):
                                            nc.tensor.matmul(
                                                ps[:],
                                                xc[:, kk, mt * 128:(mt + 1) * 128],
                                                wv_t[:, kk, :],
                                                start=(kk == 0),
                                                stop=(kk == KK - 1),
                                            )
                                        vt = V[n * 4 + mt]
                                        if kb == 0:
                                            nc.vector.tensor_copy(vt[:], ps[:])
                                        else:
                                            nc.vector.tensor_add(vt[:], vt[:], ps[:])

                        # ---------------- phase 2: attention ----------------
                        with tc.tile_pool(name=f"a2e{b}", bufs=3) as ep, \
                             tc.tile_pool(name=f"a2rb{b}", bufs=1) as rbp:
                            for h in range(HL):
                                for sq in range(4):
                                    q0 = sq * 512
                                    po = pop.tile([128, 512], fp32, tag="po")
                                    psum = psump.tile([1, 512], fp32, tag="psum")
                                    for sk in range(16):
                                        pS = pSp.tile([128, 512], fp32, tag="pS")
                                        nc.tensor.matmul(
                                            pS[:],
                                            KT[h][:, sk * 128:(sk + 1) * 128],
                                            QT[h][:, q0:q0 + 512],
                                            start=True, stop=True,
                                        )
                                        eS = ep.tile([128, 512], fp32r, tag="eS")
                                        nc.scalar.activation(eS[:], pS[:], EXP, bias=zbias[:, 0:1])
                                        nc.tensor.matmul(
                                            psum[:],
                                            ones_col[:],
                                            eS[:],
                                            start=(sk == 0), stop=(sk == 15),
                                        )
                                        nc.tensor.matmul(
                                            po[:],
                                            V[sk][:, h * 128:(h + 1) * 128],
                                            eS[:],
                                            start=(sk == 0), stop=(sk == 15),
                                        )
                                    rec = rp.tile([1, 512], fp32r, tag="rec")
                                    nc.vector.reciprocal(rec[:], psum[:])
                                    prb = gpp.tile([128, 512], fp32, tag="gp", name="prb")
                                    nc.tensor.matmul(
                                        prb[:],
                                        ones_row[:],
                                        rec[:],
                                        start=True, stop=True,
                                    )
                                    rb_sb = rbp.tile([128, 512], fp32, tag="rb_sb")
                                    nc.vector.tensor_copy(rb_sb[:], prb[:])
                                    nc.vector.tensor_mul(OT[h][:, q0:q0 + 512], po[:], rb_sb[:])

                    # ---------------- phase 3: output projection ----------------
                    for nE in range(8):
                        wo_t = wop.tile([128, KK, 512], fp32r, tag="wo")
                        nc.sync.dma_start(wo_t[:], wo_d[:, :, nE * 512:(nE + 1) * 512])
                        for m in range(16):
                            py = gpp.tile([128, 512], fp32, tag="gp", name="py")
                            for kd in range(KK):
                                nc.tensor.matmul(
                                    py[:],
                                    OT[kd][:, m * 128:(m + 1) * 128],
                                    wo_t[:, kd, :],
                                    start=(kd == 0), stop=(kd == KK - 1),
                                )
                            yt = yp.tile([128, 512], fp32, tag="yt")
                            nc.vector.tensor_copy(yt[:], py[:])
                            nc.sync.dma_start(
                                y_d[b * S + m * 128: b * S + (m + 1) * 128,
                                    nE * 512:(nE + 1) * 512],
                                yt[:],
                            )

    nc.compile()
    return nc


def _prep_inputs(x, freqs_cos, freqs_sin, wq, wk, wv, wo):
    x = np.asarray(x, np.float32)
    c = np.asarray(freqs_cos, np.float32)
    s = np.asarray(freqs_sin, np.float32)
    wq = np.asarray(wq, np.float32)
    wk = np.asarray(wk, np.float32)
    wv = np.asarray(wv, np.float32)
    wo = np.asarray(wo, np.float32)

    xT = np.ascontiguousarray(x.reshape(T, E).T)

    def fold(w):
        wr = w.reshape(H, D // 2, 2, E)
        w0, w1 = wr[:, :, 0], wr[:, :, 1]
        r0 = c[:, :, None] * w0 - s[:, :, None] * w1
        r1 = s[:, :, None] * w0 + c[:, :, None] * w1
        return np.stack([r0, r1], axis=2).reshape(E, E)

    wq_r = fold(wq) * np.float32(D ** -0.5)
    wk_r = fold(wk)

    in_maps = []
    for cix in range(NCORES):
        sl = slice(cix * W, (cix + 1) * W)
        wqT = wq_r[sl].T                      # [E, W]
        wkT = wk_r[sl].T
        qk = np.concatenate([wqT, wkT], axis=1)          # [E, 2W]
        wqk = np.ascontiguousarray(
            qk.reshape(KB, KK, 128, 2 * W).transpose(0, 2, 1, 3))
        wvb = np.ascontiguousarray(
            wv[sl].T.reshape(KB, KK, 128, W).transpose(0, 2, 1, 3))
        wob = np.ascontiguousarray(
            wo[:, sl].T.reshape(KK, 128, E).transpose(1, 0, 2))
        in_maps.append({"xT": xT, "wqk": wqk, "wv": wvb, "wo": wob})
    return in_maps


def run(x, freqs_cos, freqs_sin, wq, wk, wv, wo, trace=False, tmpdir=None):
    from concourse.bass_utils import run_bass_kernel_spmd

    if "nc" not in _CACHE:
        _CACHE["nc"] = _build_nc()
    nc = _CACHE["nc"]
    in_maps = _prep_inputs(x, freqs_cos, freqs_sin, wq, wk, wv, wo)
    res = run_bass_kernel_spmd(
        nc, in_maps, list(range(NCORES)), trace=trace, tmpdir=tmpdir
    )
    y = res.results[0]["y"]
    for r in res.results[1:]:
        y = y + r["y"]
    return np.asarray(y, np.float32).reshape(B, S, E), res


def kernel(x, start_pos=0, freqs_cos=None, freqs_sin=None,
           wq=None, wk=None, wv=None, wo=None):
    y, _ = run(x, freqs_cos, freqs_sin, wq, wk, wv, wo)
    return y


# revision 13
# speedup vs baseline: 1.1093x; 1.1093x over previous
"""Llama attention (B=2, S=2048, E=4096, H=32) on 8 trn2 NeuronCores.

Strategy (tensor-parallel over heads, 4 heads/core):
  - RoPE here is position-independent (cos/sin are [H, D/2], broadcast over
    seq), so it is a fixed per-head linear rotation folded into wq/wk on the
    host.  The 1/sqrt(D) score scale is folded into wq as well.
  - Scores are computed transposed (S^T = K^T-tile @ Q^T) and the attention
    output as O^T = V-tile @ P^T, so the device kernel is pure matmuls +
    exp with zero on-device transposes.  Softmax needs no max-subtraction
    (scores are bounded ~ +-8 here; fp32 exp cannot overflow).
  - Softmax denominators via an M=1 ones-matmul on the PE; the reciprocal is
    broadcast across partitions with a K=1 ones-matmul.
  - All matmuls run as float32r (full fp32 data, 1 cycle/row for N>=256).
  - Per-core output is a partial Y (row-sharded wo); host sums the 8 partials.
"""

import sys

sys.path.insert(0, "/opt/trn_rl_repo")

import numpy as np

B, S, E, H = 2, 2048, 4096, 32
D = 128            # head dim
NCORES = 8
HL = H // NCORES   # heads per core = 4
W = HL * D         # per-core projection width = 512
T = B * S          # 4096 tokens
KB = 8             # contraction blocks over E (512 each)
KK = 4             # 128-row k-tiles per block
NCH = 4            # 512-token chunks per batch
CH = 512

_CACHE = {}


def _build_nc():
    import concourse.bass as bass  # noqa: F401
    import concourse.mybir as mybir
    import concourse.tile as tile
    from concourse import bacc

    fp32 = mybir.dt.float32
    fp32r = mybir.dt.float32r
    EXP = mybir.ActivationFunctionType.Exp

    nc = bacc.Bacc("TRN2", target_bir_lowering=False, debug=False)

    xT_d = nc.dram_tensor("xT", [E, T], fp32r, kind="ExternalInput")
    wqk_d = nc.dram_tensor("wqk", [KB, 128, KK, 2 * W], fp32r, kind="ExternalInput")
    wv_d = nc.dram_tensor("wv", [KB, 128, KK, W], fp32r, kind="ExternalInput")
    wo_d = nc.dram_tensor("wo", [128, KK, E], fp32r, kind="ExternalInput")
    y_d = nc.dram_tensor("y", [T, E], fp32, kind="ExternalOutput")

    xview = xT_d.rearrange("(kb kk p) t -> kb p kk t", kk=KK, p=128)

    from concourse.bass_isa import ReduceOp

    with nc.allow_low_precision(reason="fp32r feeds PE; rounding is intended"), \
         tile.TileContext(nc) as tc:
        with tc.tile_pool(name="const", bufs=1) as constp, \
             tc.tile_pool(name="g_wo", bufs=2) as wop, \
             tc.tile_pool(name="g_yt", bufs=3) as yp, \
             tc.tile_pool(name="g_psA", bufs=5, space="PSUM") as psA, \
             tc.tile_pool(name="g_psB", bufs=3, space="PSUM") as psB:
            zbias = constp.tile([128, 1], fp32, tag="zbias")
            nc.vector.memset(zbias[:], 0.0)

            for b in range(B):
                with tc.tile_pool(name=f"ot{b}", bufs=1) as otp:
                    OT = [otp.tile([128, S], fp32r, tag=f"ot{i}", name=f"ot{i}") for i in range(HL)]
                    with tc.tile_pool(name=f"qkv{b}", bufs=1) as qkvp:
                        QT = [qkvp.tile([128, S], fp32r, tag=f"qt{i}", name=f"qt{i}") for i in range(HL)]
                        KT = [qkvp.tile([128, S], fp32r, tag=f"kt{i}", name=f"kt{i}") for i in range(HL)]
                        V = [qkvp.tile([128, W], fp32r, tag=f"v{i}", name=f"v{i}") for i in range(4 * NCH)]

                        # ---------------- phase 1: projections ----------------
                        with tc.tile_pool(name=f"p1w{b}", bufs=2) as wpool, \
                             tc.tile_pool(name=f"p1wv{b}", bufs=1) as wvpool, \
                             tc.tile_pool(name=f"p1x{b}", bufs=2) as xpool:
                            for kb in range(KB):
                                wqk_t = wpool.tile([128, KK, 2 * W], fp32r, tag="wqk")
                                nc.sync.dma_start(wqk_t[:], wqk_d[kb])
                                wv_t = wvpool.tile([128, KK, W], fp32r, tag="wv")
                                nc.sync.dma_start(wv_t[:], wv_d[kb])
                                for n in range(NCH):
                                    tok0 = b * S + n * CH
                                    xc = xpool.tile([128, KK, CH], fp32r, tag="xc")
                                    nc.sync.dma_start(
                                        xc[:], xview[kb, :, :, tok0:tok0 + CH]
                                    )
                                    for proj in range(2):  # 0 -> QT, 1 -> KT
                                        for mi in range(HL):
                                            ps = psA.tile([128, CH], fp32, tag="psA", name="ps")
                                            c0 = proj * W + mi * 128
                                            for kk in range(KK):
                                                nc.tensor.matmul(
                                                    ps[:],
                                                    wqk_t[:, kk, c0:c0 + 128],
                                                    xc[:, kk, :],
                                                    start=(kk == 0),
                                                    stop=(kk == KK - 1),
                                                )
                                            dst = (QT if proj == 0 else KT)[mi][:, n * CH:(n + 1) * CH]
                                            if kb == 0:
                                                nc.vector.tensor_copy(dst, ps[:])
                                            else:
                                                nc.vector.tensor_add(dst, dst, ps[:])
                                    for mt in range(4):  # V: token tiles in chunk
                                        ps = psA.tile([128, W], fp32, tag="psA", name="psv")
                                        for kk in range(KK):
                                            nc.tensor.matmul(
                                                ps[:],
                                                xc[:, kk, mt * 128:(mt + 1) * 128],
                                                wv_t[:, kk, :],
                                                start=(kk == 0),
                                                stop=(kk == KK - 1),
                                            )
                                        vt = V[n * 4 + mt]
                                        if kb == 0:
                                            nc.vector.tensor_copy(vt[:], ps[:])
                                        else:
                                            nc.vector.tensor_add(vt[:], vt[:], ps[:])

                        # ---------------- phase 2: attention ----------------
                        with tc.tile_pool(name=f"a2e{b}", bufs=4) as ep, \
                             tc.tile_pool(name=f"a2s{b}", bufs=2) as esp:
                            for h in range(HL):
                                for sq in range(4):
                                    q0 = sq * 512
                                    po = psB.tile([128, 512], fp32, tag="psB", name="po")
                                    esum = esp.tile([128, 512], fp32, tag="esum")
                                    for sk in range(16):
                                        pS = psA.tile([128, 512], fp32, tag="psA", name="pS")
                                        nc.tensor.matmul(
                                            pS[:],
                                            KT[h][:, sk * 128:(sk + 1) * 128],
                                            QT[h][:, q0:q0 + 512],
                                            start=True, stop=True,
                                        )
                                        eS = ep.tile([128, 512], fp32r, tag="eS")
                                        nc.scalar.activation(eS[:], pS[:], EXP, bias=zbias[:, 0:1])
                                        nc.tensor.matmul(
                                            po[:],
                                            V[sk][:, h * 128:(h + 1) * 128],
                                            eS[:],
                                            start=(sk == 0), stop=(sk == 15),
                                        )
                                        if sk == 0:
                                            nc.vector.tensor_copy(esum[:], eS[:])
                                        else:
                                            nc.vector.tensor_add(esum[:], esum[:], eS[:])
                                    nc.gpsimd.partition_all_reduce(
                                        esum[:], esum[:], 128, ReduceOp.add)
                                    nc.vector.reciprocal(esum[:], esum[:])
                                    nc.vector.tensor_mul(OT[h][:, q0:q0 + 512], po[:], esum[:])

                    # ---------------- phase 3: output projection ----------------
                    for nE in range(8):
                        wo_t = wop.tile([128, KK, 512], fp32r, tag="wo")
                        nc.sync.dma_start(wo_t[:], wo_d[:, :, nE * 512:(nE + 1) * 512])
                        for m in range(16):
                            py = psB.tile([128, 512], fp32, tag="psB", name="py")
                            for kd in range(KK):
                                nc.tensor.matmul(
                                    py[:],
                                    OT[kd][:, m * 128:(m + 1) * 128],
                                    wo_t[:, kd, :],
                                    start=(kd == 0), stop=(kd == KK - 1),
                                )
                            yt = yp.tile([128, 512], fp32, tag="yt")
                            nc.vector.tensor_copy(yt[:], py[:])
                            nc.sync.dma_start(
                                y_d[b * S + m * 128: b * S + (m + 1) * 128,
                                    nE * 512:(nE + 1) * 512],
                                yt[:],
                            )

    nc.compile()
    return nc


def _prep_inputs(x, freqs_cos, freqs_sin, wq, wk, wv, wo):
    x = np.asarray(x, np.float32)
    c = np.asarray(freqs_cos, np.float32)
    s = np.asarray(freqs_sin, np.float32)
    wq = np.asarray(wq, np.float32)
    wk = np.asarray(wk, np.float32)
    wv = np.asarray(wv, np.float32)
    wo = np.asarray(wo, np.float32)

    xT = np.ascontiguousarray(x.reshape(T, E).T)

    def fold(w):
        wr = w.reshape(H, D // 2, 2, E)
        w0, w1 = wr[:, :, 0], wr[:, :, 1]
        r0 = c[:, :, None] * w0 - s[:, :, None] * w1
        r1 = s[:, :, None] * w0 + c[:, :, None] * w1
        return np.stack([r0, r1], axis=2).reshape(E, E)

    wq_r = fold(wq) * np.float32(D ** -0.5)
    wk_r = fold(wk)

    in_maps = []
    for cix in range(NCORES):
        sl = slice(cix * W, (cix + 1) * W)
        wqT = wq_r[sl].T                      # [E, W]
        wkT = wk_r[sl].T
        qk = np.concatenate([wqT, wkT], axis=1)          # [E, 2W]
        wqk = np.ascontiguousarray(
            qk.reshape(KB, KK, 128, 2 * W).transpose(0, 2, 1, 3))
        wvb = np.ascontiguousarray(
            wv[sl].T.reshape(KB, KK, 128, W).transpose(0, 2, 1, 3))
        wob = np.ascontiguousarray(
            wo[:, sl].T.reshape(KK, 128, E).transpose(1, 0, 2))
        in_maps.append({"xT": xT, "wqk": wqk, "wv": wvb, "wo": wob})
    return in_maps


def run(x, freqs_cos, freqs_sin, wq, wk, wv, wo, trace=False, tmpdir=None):
    from concourse.bass_utils import run_bass_kernel_spmd

    if "nc" not in _CACHE:
        _CACHE["nc"] = _build_nc()
    nc = _CACHE["nc"]
    in_maps = _prep_inputs(x, freqs_cos, freqs_sin, wq, wk, wv, wo)
    res = run_bass_kernel_spmd(
        nc, in_maps, list(range(NCORES)), trace=trace, tmpdir=tmpdir
    )
    y = res.results[0]["y"]
    for r in res.results[1:]:
        y = y + r["y"]
    return np.asarray(y, np.float32).reshape(B, S, E), res


def kernel(x, start_pos=0, freqs_cos=None, freqs_sin=None,
           wq=None, wk=None, wv=None, wo=None):
    y, _ = run(x, freqs_cos, freqs_sin, wq, wk, wv, wo)
    return y


# revision 15
# speedup vs baseline: 1.1275x; 1.0164x over previous
"""Llama attention (B=2, S=2048, E=4096, H=32) on 8 trn2 NeuronCores.

Strategy (tensor-parallel over heads, 4 heads/core):
  - RoPE here is position-independent (cos/sin are [H, D/2], broadcast over
    seq), so it is a fixed per-head linear rotation folded into wq/wk on the
    host.  The 1/sqrt(D) score scale is folded into wq as well.
  - Scores are computed transposed (S^T = K^T-tile @ Q^T) and the attention
    output as O^T = V-tile @ P^T, so the device kernel is pure matmuls +
    exp with zero on-device transposes.  Softmax needs no max-subtraction
    (scores are bounded ~ +-8 here; fp32 exp cannot overflow).
  - Softmax denominators via an M=1 ones-matmul on the PE; the reciprocal is
    broadcast across partitions with a K=1 ones-matmul.
  - All matmuls run as float32r (full fp32 data, 1 cycle/row for N>=256).
  - Per-core output is a partial Y (row-sharded wo); host sums the 8 partials.
"""

import sys

sys.path.insert(0, "/opt/trn_rl_repo")

import numpy as np

B, S, E, H = 2, 2048, 4096, 32
D = 128            # head dim
NCORES = 8
HL = H // NCORES   # heads per core = 4
W = HL * D         # per-core projection width = 512
T = B * S          # 4096 tokens
KB = 8             # contraction blocks over E (512 each)
KK = 4             # 128-row k-tiles per block
NCH = 4            # 512-token chunks per batch
CH = 512

_CACHE = {}


def _build_nc():
    import concourse.bass as bass  # noqa: F401
    import concourse.mybir as mybir
    import concourse.tile as tile
    from concourse import bacc

    fp32 = mybir.dt.float32
    fp32r = mybir.dt.float32r
    EXP = mybir.ActivationFunctionType.Exp

    nc = bacc.Bacc("TRN2", target_bir_lowering=False, debug=False)

    xT_d = nc.dram_tensor("xT", [E, T], fp32r, kind="ExternalInput")
    wqk_d = nc.dram_tensor("wqk", [KB, 128, KK, 2 * W], fp32r, kind="ExternalInput")
    wv_d = nc.dram_tensor("wv", [KB, 128, KK, W], fp32r, kind="ExternalInput")
    wo_d = nc.dram_tensor("wo", [128, KK, E], fp32r, kind="ExternalInput")
    y_d = nc.dram_tensor("y", [T, E], fp32, kind="ExternalOutput")

    xview = xT_d.rearrange("(kb kk p) t -> kb p kk t", kk=KK, p=128)

    from concourse.bass_isa import ReduceOp

    with nc.allow_low_precision(reason="fp32r feeds PE; rounding is intended"), \
         tile.TileContext(nc) as tc:
        with tc.tile_pool(name="const", bufs=1) as constp, \
             tc.tile_pool(name="g_wo", bufs=2) as wop, \
             tc.tile_pool(name="g_yt", bufs=3) as yp, \
             tc.tile_pool(name="g_psA", bufs=5, space="PSUM") as psA, \
             tc.tile_pool(name="g_psB", bufs=3, space="PSUM") as psB:
            zbias = constp.tile([128, 1], fp32, tag="zbias")
            nc.vector.memset(zbias[:], 0.0)

            for b in range(B):
                with tc.tile_pool(name=f"ot{b}", bufs=1) as otp:
                    OT = [otp.tile([128, S], fp32r, tag=f"ot{i}", name=f"ot{i}") for i in range(HL)]
                    with tc.tile_pool(name=f"qkv{b}", bufs=1) as qkvp:
                        QT = [qkvp.tile([128, S], fp32r, tag=f"qt{i}", name=f"qt{i}") for i in range(HL)]
                        KT = [qkvp.tile([128, S], fp32r, tag=f"kt{i}", name=f"kt{i}") for i in range(HL)]
                        V = [qkvp.tile([128, W], fp32r, tag=f"v{i}", name=f"v{i}") for i in range(4 * NCH)]

                        # ---------------- phase 1: projections ----------------
                        with tc.tile_pool(name=f"p1w{b}", bufs=2) as wpool, \
                             tc.tile_pool(name=f"p1wv{b}", bufs=1) as wvpool, \
                             tc.tile_pool(name=f"p1x{b}", bufs=2) as xpool:
                            for kb in range(KB):
                                wqk_t = wpool.tile([128, KK, 2 * W], fp32r, tag="wqk")
                                nc.sync.dma_start(wqk_t[:], wqk_d[kb])
                                wv_t = wvpool.tile([128, KK, W], fp32r, tag="wv")
                                nc.sync.dma_start(wv_t[:], wv_d[kb])
                                for n in range(NCH):
                                    tok0 = b * S + n * CH
                                    xc = xpool.tile([128, KK, CH], fp32r, tag="xc")
                                    nc.sync.dma_start(
                                        xc[:], xview[kb, :, :, tok0:tok0 + CH]
                                    )
                                    for proj in range(2):  # 0 -> QT, 1 -> KT
                                        for mi in range(HL):
                                            ps = psA.tile([128, CH], fp32, tag="psA", name="ps")
                                            c0 = proj * W + mi * 128
                                            for kk in range(KK):
                                                nc.tensor.matmul(
                                                    ps[:],
                                                    wqk_t[:, kk, c0:c0 + 128],
                                                    xc[:, kk, :],
                                                    start=(kk == 0),
                                                    stop=(kk == KK - 1),
                                                )
                                            dst = (QT if proj == 0 else KT)[mi][:, n * CH:(n + 1) * CH]
                                            if kb == 0:
                                                nc.vector.tensor_copy(dst, ps[:])
                                            else:
                                                nc.vector.tensor_add(dst, dst, ps[:])
                                    for mt in range(4):  # V: token tiles in chunk
                                        ps = psA.tile([128, W], fp32, tag="psA", name="psv")
                                        for kk in range(KK):
                                            nc.tensor.matmul(
                                                ps[:],
                                                xc[:, kk, mt * 128:(mt + 1) * 128],
                                                wv_t[:, kk, :],
                                                start=(kk == 0),
                                                stop=(kk == KK - 1),
                                            )
                                        vt = V[n * 4 + mt]
                                        if kb == 0:
                                            nc.vector.tensor_copy(vt[:], ps[:])
                                        else:
                                            nc.vector.tensor_add(vt[:], vt[:], ps[:])

                        # ---------------- phase 2: attention ----------------
                        with tc.tile_pool(name=f"a2e{b}", bufs=4) as ep, \
                             tc.tile_pool(name=f"a2s{b}", bufs=2) as esp, \
                             tc.tile_pool(name=f"a2r{b}", bufs=2) as rcp:
                            for h in range(HL):
                                RECIP = rcp.tile([128, S], fp32, tag="recip", name="recip")
                                for sq in range(4):
                                    q0 = sq * 512
                                    po = psB.tile([128, 512], fp32, tag="psB", name="po")
                                    esumA = esp.tile([128, 512], fp32, tag="esumA")
                                    esumB = esp.tile([128, 512], fp32, tag="esumB")
                                    for sk in range(16):
                                        pS = psA.tile([128, 512], fp32, tag="psA", name="pS")
                                        nc.tensor.matmul(
                                            pS[:],
                                            KT[h][:, sk * 128:(sk + 1) * 128],
                                            QT[h][:, q0:q0 + 512],
                                            start=True, stop=True,
                                        )
                                        eS = ep.tile([128, 512], fp32r, tag="eS")
                                        nc.scalar.activation(eS[:], pS[:], EXP, bias=zbias[:, 0:1])
                                        nc.tensor.matmul(
                                            po[:],
                                            V[sk][:, h * 128:(h + 1) * 128],
                                            eS[:],
                                            start=(sk == 0), stop=(sk == 15),
                                        )
                                        dst = esumA if sk % 2 == 0 else esumB
                                        if sk < 2:
                                            nc.vector.tensor_copy(dst[:], eS[:])
                                        else:
                                            nc.vector.tensor_add(dst[:], dst[:], eS[:])
                                    # raw (unnormalized) O^T out of PSUM fast
                                    nc.vector.tensor_copy(OT[h][:, q0:q0 + 512], po[:])
                                    nc.vector.tensor_add(esumA[:], esumA[:], esumB[:])
                                    nc.gpsimd.partition_all_reduce(
                                        esumA[:], esumA[:], 128, ReduceOp.add)
                                    nc.vector.reciprocal(
                                        RECIP[:, q0:q0 + 512], esumA[:])
                                # normalize the whole head in one sweep
                                nc.vector.tensor_mul(OT[h][:], OT[h][:], RECIP[:])

                    # ---------------- phase 3: output projection ----------------
                    for nE in range(8):
                        wo_t = wop.tile([128, KK, 512], fp32r, tag="wo")
                        nc.sync.dma_start(wo_t[:], wo_d[:, :, nE * 512:(nE + 1) * 512])
                        for m in range(16):
                            py = psB.tile([128, 512], fp32, tag="psB", name="py")
                            for kd in range(KK):
                                nc.tensor.matmul(
                                    py[:],
                                    OT[kd][:, m * 128:(m + 1) * 128],
                                    wo_t[:, kd, :],
                                    start=(kd == 0), stop=(kd == KK - 1),
                                )
                            yt = yp.tile([128, 512], fp32, tag="yt")
                            nc.vector.tensor_copy(yt[:], py[:])
                            nc.sync.dma_start(
                                y_d[b * S + m * 128: b * S + (m + 1) * 128,
                                    nE * 512:(nE + 1) * 512],
                                yt[:],
                            )

    nc.compile()
    return nc


def _prep_inputs(x, freqs_cos, freqs_sin, wq, wk, wv, wo):
    x = np.asarray(x, np.float32)
    c = np.asarray(freqs_cos, np.float32)
    s = np.asarray(freqs_sin, np.float32)
    wq = np.asarray(wq, np.float32)
    wk = np.asarray(wk, np.float32)
    wv = np.asarray(wv, np.float32)
    wo = np.asarray(wo, np.float32)

    xT = np.ascontiguousarray(x.reshape(T, E).T)

    def fold(w):
        wr = w.reshape(H, D // 2, 2, E)
        w0, w1 = wr[:, :, 0], wr[:, :, 1]
        r0 = c[:, :, None] * w0 - s[:, :, None] * w1
        r1 = s[:, :, None] * w0 + c[:, :, None] * w1
        return np.stack([r0, r1], axis=2).reshape(E, E)

    wq_r = fold(wq) * np.float32(D ** -0.5)
    wk_r = fold(wk)

    in_maps = []
    for cix in range(NCORES):
        sl = slice(cix * W, (cix + 1) * W)
        wqT = wq_r[sl].T                      # [E, W]
        wkT = wk_r[sl].T
        qk = np.concatenate([wqT, wkT], axis=1)          # [E, 2W]
        wqk = np.ascontiguousarray(
            qk.reshape(KB, KK, 128, 2 * W).transpose(0, 2, 1, 3))
        wvb = np.ascontiguousarray(
            wv[sl].T.reshape(KB, KK, 128, W).transpose(0, 2, 1, 3))
        wob = np.ascontiguousarray(
            wo[:, sl].T.reshape(KK, 128, E).transpose(1, 0, 2))
        in_maps.append({"xT": xT, "wqk": wqk, "wv": wvb, "wo": wob})
    return in_maps


def run(x, freqs_cos, freqs_sin, wq, wk, wv, wo, trace=False, tmpdir=None):
    from concourse.bass_utils import run_bass_kernel_spmd

    if "nc" not in _CACHE:
        _CACHE["nc"] = _build_nc()
    nc = _CACHE["nc"]
    in_maps = _prep_inputs(x, freqs_cos, freqs_sin, wq, wk, wv, wo)
    res = run_bass_kernel_spmd(
        nc, in_maps, list(range(NCORES)), trace=trace, tmpdir=tmpdir
    )
    y = res.results[0]["y"]
    for r in res.results[1:]:
        y = y + r["y"]
    return np.asarray(y, np.float32).reshape(B, S, E), res


def kernel(x, start_pos=0, freqs_cos=None, freqs_sin=None,
           wq=None, wk=None, wv=None, wo=None):
    y, _ = run(x, freqs_cos, freqs_sin, wq, wk, wv, wo)
    return y
